# revision 15
# baseline (speedup 1.0000x reference)
"""BlockSparseAttention TRN2 kernel — 8-core SPMD (batch x head-half sharding).

Reference computation (B=4, S=2048, D=1024, H=16, Dh=64):
  q/k/v = x @ W{q,k,v}.T + b   -> [B,H,S,Dh]
  scores = q k^T / 8, masked to |i-j|<=32 plus global rows/cols (<4)
  out = softmax(scores) v  -> reassembled -> @ Wo.T + bo

Sharding: core c handles batch b=c//2, head-group g=c%2 (heads 8g..8g+7,
channels 512g..512g+511). Each core computes its heads' attention output and
a PARTIAL o-projection (contraction over its 512 channels); host sums the two
partials per batch and adds bo.

On-chip layout is fully transposed (d-major): projections compute q_T/k_T
directly as [chan, s]; scores are computed transposed [t, s] so softmax sums
land in a matmul ones-row (v augmented with a ones column).

v2 design notes (vs v1):
  - x is SBUF-resident, DMA'd once; projections run cb-outer so q/k/v fit in
    a 4-bank PSUM rotation and form one continuous dense PE stream.
  - Attention is a 1-deep software pipeline over the 32 (r, hp) pairs:
    scores(j) | bc(j-2) | AV(j-1) per iteration, with PE work grouped by
    tiling mode (64-row scores w/ h0/h64 row-tile concurrency, 32-row
    broadcast MMs, 128-row AVs) to minimize PE array mode-switch drains.
  - Global (t<4) key columns are computed as a 5th band-style chunk with a
    0/1 bf16 mask MULTIPLY after exp (no -1e9 add pass, no 4-partition strip
    matmuls, no 32-row AV section).
  - Softmax normalization: denominator row -> bf16 copy -> PE K=1 broadcast
    matmul (col-tiled into the att layout) -> reciprocal_approx_fast on the
    [128, 256] broadcast -> one [128, 256] multiply. No GpSimd broadcast.
  - o-projection emitted as dense N=512 blocks after the norm of each s-tile
    pair; output stored bf16 (host sums partials in f32).
"""
import numpy as np
import ml_dtypes

import concourse.bass as bass
import concourse.bacc as bacc
import concourse.tile as tile
import concourse.mybir as mybir
from concourse.bass_utils import run_bass_kernel_spmd

F32 = mybir.dt.float32
BF16 = mybir.dt.bfloat16
AF = mybir.ActivationFunctionType
ALU = mybir.AluOpType

S = 2048
D = 1024
NCORES = 8
SCALE = 0.125

# ---------------------------------------------------------------------------
# Chunk plans: per s-tile r (256 rows), the score/AV chunks.
# Each chunk: (sc, exoff, lo, w, mkind) where sc = 128-row v/t chunk index,
# exoff = column offset in the packed ex tile, lo = s-local start, w = width,
# mkind identifies the mask pattern.
#   mkind: 'G' (global cols t<4), 'D' (delta=-128), 'A' (delta=0),
#          'B' (delta=+128), 'C' (delta=+256), 'S' (r=0 special j0)
# sctA holds the first 448 ex columns, sctB the rest (<=192).
# ---------------------------------------------------------------------------


def chunk_plan(r):
    if r == 0:
        return [(0, 0, 0, 256, 'S'), (1, 256, 96, 160, 'B'),
                (2, 416, 224, 32, 'C')]
    if r == 7:
        return [(0, 0, 0, 256, 'G'), (13, 256, 0, 32, 'D'),
                (14, 288, 0, 160, 'A'), (15, 448, 96, 160, 'B')]
    return [(0, 0, 0, 256, 'G'), (2 * r - 1, 256, 0, 32, 'D'),
            (2 * r, 288, 0, 160, 'A'), (2 * r + 1, 448, 96, 160, 'B'),
            (2 * r + 2, 608, 224, 32, 'C')]


CHUNKS = [chunk_plan(r) for r in range(8)]
WR = [sum(c[3] for c in CHUNKS[r]) for r in range(8)]  # 448 / 640 / 608
A_COLS = 448
# mask packing: one column-block per r-class, matching ex layout
MOFF = {0: 640, 7: 1088}  # interior at 0
MASK_W = 640 + 448 + 608


def mask_off(r):
    return MOFF.get(r, 0)


def build_nc():
    nc = bacc.Bacc()
    xT = nc.dram_tensor("xT", [128, 8, S], BF16, kind="ExternalInput")
    wq = nc.dram_tensor("wq", [128, 8, 512], BF16, kind="ExternalInput")
    wk = nc.dram_tensor("wk", [128, 8, 512], BF16, kind="ExternalInput")
    wv = nc.dram_tensor("wv", [128, 8, 512], BF16, kind="ExternalInput")
    wo = nc.dram_tensor("wo", [128, 4, 1024], BF16, kind="ExternalInput")
    bq_c = nc.dram_tensor("bq_c", [128, 4], F32, kind="ExternalInput")
    bk_c = nc.dram_tensor("bk_c", [128, 4], F32, kind="ExternalInput")
    bv_b = nc.dram_tensor("bv_b", [128, 512], F32, kind="ExternalInput")
    masks = nc.dram_tensor("masks", [128, MASK_W], BF16, kind="ExternalInput")
    out = nc.dram_tensor("out", [128, 8, S], BF16, kind="ExternalOutput")

    with tile.TileContext(nc) as tc:
        with (
            tc.tile_pool(name="pers", bufs=1) as pers,
            tc.tile_pool(name="small", bufs=1) as small,
        ):
            x_sb = pers.tile([128, 8, S], BF16)
            q_sb = pers.tile([128, 4, S], BF16)
            k_sb = pers.tile([128, 4, S], BF16)
            v_sb = pers.tile([128, 16, 520], BF16)
            att_sb = pers.tile([128, 4, S], BF16)
            masks_sb = pers.tile([128, MASK_W], BF16)
            wq_sb = pers.tile([128, 8, 512], BF16)
            wk_sb = pers.tile([128, 8, 512], BF16)
            wv_sb = pers.tile([128, 8, 512], BF16)
            wo_sb = pers.tile([128, 4, 1024], BF16)
            bq_sb = small.tile([128, 4], F32)
            bk_sb = small.tile([128, 4], F32)
            bv_sb = small.tile([128, 512], F32)
            ones_sb = small.tile([1, 128], BF16)
            nc.vector.memset(ones_sb, 1.0)

            # Two HWDGE queues in parallel: weights/biases/masks on the
            # scalar-engine queue, x on the sync queue — first q chain can
            # start once wq and x-st0 land (~6us).
            nc.scalar.dma_start(out=wq_sb, in_=wq.ap())
            nc.scalar.dma_start(out=bq_sb, in_=bq_c.ap())
            nc.scalar.dma_start(out=bk_sb, in_=bk_c.ap())
            nc.scalar.dma_start(out=wk_sb, in_=wk.ap())
            nc.scalar.dma_start(out=wv_sb, in_=wv.ap())
            nc.scalar.dma_start(out=bv_sb, in_=bv_b.ap())
            nc.scalar.dma_start(out=masks_sb, in_=masks.ap())
            nc.scalar.dma_start(out=wo_sb, in_=wo.ap())
            for st in range(4):
                ssl = slice(st * 512, (st + 1) * 512)
                for dc in range(8):
                    nc.sync.dma_start(out=x_sb[:, dc, ssl], in_=xT.ap()[:, dc, ssl])

            # ---------------- Phase 1: projections (dense, x-resident) -----
            # st=0..2 here; st=3's 12 chains are deferred into the early
            # attention iterations as dense HAM-warming filler.
            def proj_chain(pool, kind, st, idx):
                ssl = slice(st * 512, (st + 1) * 512)
                if kind == 'q' or kind == 'k':
                    csl = slice(idx * 128, (idx + 1) * 128)
                    w_sb = wq_sb if kind == 'q' else wk_sb
                    dst = q_sb if kind == 'q' else k_sb
                    b_sb = bq_sb if kind == 'q' else bk_sb
                    pq = pool.tile([128, 512], F32, tag=pool_tag(pool),
                                   name=f"p{kind}{st}_{idx}")
                    for dc in range(8):
                        nc.tensor.matmul(
                            pq, w_sb[:, dc, csl], x_sb[:, dc, ssl],
                            start=(dc == 0), stop=(dc == 7))
                    nc.vector.tensor_scalar(
                        out=dst[:, idx, ssl], in0=pq,
                        scalar1=b_sb[:, idx:idx + 1], scalar2=None,
                        op0=ALU.add)
                else:  # 'v'
                    xsl = slice(st * 512 + idx * 128, st * 512 + idx * 128 + 128)
                    pv = pool.tile([128, 512], F32, tag=pool_tag(pool),
                                   name=f"pv{st}_{idx}")
                    for dc in range(8):
                        nc.tensor.matmul(
                            pv, x_sb[:, dc, xsl], wv_sb[:, dc, :],
                            start=(dc == 0), stop=(dc == 7))
                    sc = st * 4 + idx
                    vview = v_sb[:, sc, :].rearrange("p (h w) -> p h w", h=8)
                    nc.vector.tensor_add(
                        out=vview[:, :, 0:64],
                        in0=pv.rearrange("p (h w) -> p h w", h=8),
                        in1=bv_sb.rearrange("p (h w) -> p h w", h=8))
                    nc.vector.memset(vview[:, :, 64:65], 1.0)

            def pool_tag(pool):
                return "proj" if pool is not None and pool.name == "pproj" \
                    else "pobc"

            with tc.tile_pool(name="pproj", bufs=4, space="PSUM") as pproj:
                for st in range(3):
                    for cb in range(4):
                        proj_chain(pproj, 'q', st, cb)
                    for cb in range(4):
                        proj_chain(pproj, 'k', st, cb)
                    for s4 in range(4):
                        proj_chain(pproj, 'v', st, s4)

            # ---------------- Phase 2: attention pipeline ------------------
            with (
                tc.tile_pool(name="psctA", bufs=2, space="PSUM") as psctA,
                tc.tile_pool(name="psctB", bufs=2, space="PSUM") as psctB,
                tc.tile_pool(name="paug", bufs=2, space="PSUM") as paug,
                # bc (normalizer broadcast) and po (o-proj) share one
                # 2-slot rotation: 8 PSUM banks total, and their lifetimes
                # interleave without stalls.
                tc.tile_pool(name="pmix", bufs=2, space="PSUM") as pmix,
                tc.tile_pool(name="epool", bufs=4) as epool,
                tc.tile_pool(name="rpool", bufs=2) as rpool,
                tc.tile_pool(name="opool", bufs=3) as opool,
            ):
                # r=0 needs full k (gsc) and full v (gAV): schedule it after
                # r=1..3 so the deferred st=3 chains (emitted in iterations
                # 0-11) complete first.
                PAIRS = [(r, hp) for r in (1, 2, 3, 0, 4, 5, 6, 7)
                         for hp in range(4)]
                # deferred st=3 projection chains: one per early iteration
                DEFERRED = [('k', cb) for cb in range(4)] + \
                    [('q', cb) for cb in range(4)] + \
                    [('v', s4) for s4 in range(4)]
                # oproj availability: st1 after iter 13 (r2,r3 normed),
                # st0 after 17, st2 after 25, st3 in flush
                OSCHED = {}
                for i, e in enumerate(range(13, 17)):
                    OSCHED[e] = [(1, i)]
                for i, e in enumerate(range(17, 21)):
                    OSCHED[e] = [(0, i)]
                for i, e in enumerate(range(21, 25)):
                    OSCHED[e] = [(1, 4 + i)]
                for i, e in enumerate(range(25, 29)):
                    OSCHED[e] = [(0, 4 + i), (2, i)]
                OSCHED[29] = [(2, 4), (2, 5)]
                OSCHED[30] = [(2, 6), (2, 7)]
                state = {}  # j -> dict of tiles

                def emit_scores(j):
                    r, hp = PAIRS[j]
                    st = {}
                    b_cols = WR[r] - A_COLS
                    for hs in (0, 64):
                        st[f"sctA{hs}"] = psctA.tile(
                            [128, 512], F32, tag="sctA", name=f"sA{j}_{hs}")
                        if b_cols > 0:
                            st[f"sctB{hs}"] = psctB.tile(
                                [128, 192], F32, tag="sctB", name=f"sB{j}_{hs}")
                    # interleave h0/h64 chunk by chunk: row-tile concurrency
                    ca = [c for c in CHUNKS[r] if c[1] < A_COLS]
                    cbl = [c for c in CHUNKS[r] if c[1] >= A_COLS]
                    for ci, (sc, exoff, lo, w, mk) in enumerate(ca):
                        for hs in (0, 64):
                            nc.tensor.matmul(
                                st[f"sctA{hs}"][:, exoff:exoff + w],
                                k_sb[hs:hs + 64, hp, sc * 128:sc * 128 + 128],
                                q_sb[hs:hs + 64, hp,
                                     r * 256 + lo:r * 256 + lo + w],
                                start=(ci == 0), stop=(ci == len(ca) - 1),
                                skip_group_check=True)
                    for ci, (sc, exoff, lo, w, mk) in enumerate(cbl):
                        off = exoff - A_COLS
                        for hs in (0, 64):
                            nc.tensor.matmul(
                                st[f"sctB{hs}"][:, off:off + w],
                                k_sb[hs:hs + 64, hp, sc * 128:sc * 128 + 128],
                                q_sb[hs:hs + 64, hp,
                                     r * 256 + lo:r * 256 + lo + w],
                                start=(ci == 0), stop=(ci == len(cbl) - 1),
                                skip_group_check=True)
                    if r == 0:
                        for hs in (0, 64):
                            st[f"gsc{hs}"] = psctB.tile(
                                [128, 64], F32, tag="sctB", name=f"gs{j}_{hs}")
                        for kk in range(16):
                            for hs in (0, 64):
                                nc.tensor.matmul(
                                    st[f"gsc{hs}"][:, 4 * kk:4 * kk + 4],
                                    k_sb[hs:hs + 64, hp,
                                         128 * kk:128 * kk + 128],
                                    q_sb[hs:hs + 64, hp, 0:4],
                                    start=(kk == 0), stop=(kk == 15),
                                    skip_group_check=True)
                    state[j] = st

                def emit_exps(j):
                    r, hp = PAIRS[j]
                    st = state[j]
                    wr = WR[r]
                    b_cols = wr - A_COLS
                    for hs in (0, 64):
                        ex = epool.tile([128, 640], BF16, tag="ex",
                                        name=f"ex{j}_{hs}")
                        st[f"ex{hs}"] = ex
                        nc.scalar.activation(
                            ex[:, 0:A_COLS], st[f"sctA{hs}"][:, 0:A_COLS],
                            AF.Exp, scale=SCALE)
                        if b_cols > 0:
                            nc.scalar.activation(
                                ex[:, A_COLS:wr], st[f"sctB{hs}"][:, 0:b_cols],
                                AF.Exp, scale=SCALE)
                        if r == 0:
                            exg = epool.tile([128, 64], BF16, tag="exg",
                                             name=f"xg{j}_{hs}")
                            st[f"exg{hs}"] = exg
                            nc.scalar.activation(exg, st[f"gsc{hs}"],
                                                 AF.Exp, scale=SCALE)

                def emit_maskmuls(j):
                    r, hp = PAIRS[j]
                    st = state[j]
                    wr = WR[r]
                    mo = mask_off(r)
                    for hs in (0, 64):
                        nc.vector.tensor_mul(
                            out=st[f"ex{hs}"][:, 0:wr],
                            in0=st[f"ex{hs}"][:, 0:wr],
                            in1=masks_sb[:, mo:mo + wr])

                def emit_avs(j):
                    r, hp = PAIRS[j]
                    st = state[j]
                    aug = paug.tile([65, 512], F32, tag="aug", name=f"au{j}")
                    st["aug"] = aug
                    n_ch = len(CHUNKS[r])
                    for hi, hs in enumerate((0, 64)):
                        half = (hs // 64) * 256
                        h65 = (hp * 2 + hs // 64) * 65
                        ex = st[f"ex{hs}"]
                        for ci, (sc, exoff, lo, w, mk) in enumerate(CHUNKS[r]):
                            last = (r != 0 and hi == 1 and ci == n_ch - 1)
                            nc.tensor.matmul(
                                aug[:, half + lo:half + lo + w],
                                v_sb[:, sc, h65:h65 + 65],
                                ex[:, exoff:exoff + w],
                                start=(hi == 0 and ci == 0), stop=last,
                                skip_group_check=True)
                    if r == 0:
                        for hi, hs in enumerate((0, 64)):
                            half = (hs // 64) * 256
                            h65 = (hp * 2 + hs // 64) * 65
                            exg = st[f"exg{hs}"]
                            for kk in range(16):
                                nc.tensor.matmul(
                                    aug[:, half:half + 4],
                                    v_sb[:, kk, h65:h65 + 65],
                                    exg[:, 4 * kk:4 * kk + 4],
                                    start=False,
                                    stop=(hi == 1 and kk == 15),
                                    skip_group_check=True)

                def emit_den(j):
                    st = state[j]
                    den = epool.tile([1, 512], BF16, tag="den", name=f"dn{j}")
                    st["den"] = den
                    nc.vector.tensor_copy(out=den, in_=st["aug"][64:65, :])

                def emit_attcopies(j):
                    r, hp = PAIRS[j]
                    st = state[j]
                    rsl = slice(r * 256, (r + 1) * 256)
                    for hs in (0, 64):
                        half = (hs // 64) * 256
                        nc.scalar.copy(out=att_sb[hs:hs + 64, hp, rsl],
                                       in_=st["aug"][0:64, half:half + 256])

                def emit_bc(j):
                    st = state[j]
                    bc = pmix.tile([128, 256], F32, tag="pobc", name=f"bc{j}")
                    st["bc"] = bc
                    den = st["den"]
                    nc.tensor.matmul(bc[0:64, :], ones_sb[0:1, 0:64],
                                     den[0:1, 0:256], start=True, stop=True)
                    nc.tensor.matmul(bc[64:128, :], ones_sb[0:1, 0:64],
                                     den[0:1, 256:512], start=True, stop=True,
                                     tile_position=(0, 64))
                    del st["den"]

                def emit_recmul(j):
                    r, hp = PAIRS[j]
                    st = state[j]
                    rsl = slice(r * 256, (r + 1) * 256)
                    rec = rpool.tile([128, 256], F32, tag="rec", name=f"rc{j}")
                    nc.vector.reciprocal_approx_fast(out=rec, in_=st["bc"])
                    nc.vector.tensor_mul(
                        out=att_sb[:, hp, rsl], in0=att_sb[:, hp, rsl],
                        in1=rec)
                    state.pop(j, None)

                def oproj_unit(stq, et):
                    # one dense 4-MM N=512 chain: HAM-warming filler spread
                    # through the attention pipeline
                    ssl = slice(stq * 512, (stq + 1) * 512)
                    esl = slice(et * 128, (et + 1) * 128)
                    po = pmix.tile([128, 512], F32, tag="pobc",
                                   name=f"po{stq}_{et}")
                    for cb in range(4):
                        nc.tensor.matmul(
                            po, wo_sb[:, cb, esl], att_sb[:, cb, ssl],
                            start=(cb == 0), stop=(cb == 3))
                    otq = opool.tile([128, 512], BF16, tag="otq",
                                     name=f"otq{stq}_{et}")
                    nc.vector.tensor_copy(out=otq, in_=po)
                    nc.sync.dma_start(out=out.ap()[:, et, ssl], in_=otq)

                for j in range(32):
                    emit_scores(j)
                    if j >= 2:
                        emit_bc(j - 2)
                    if j >= 1:
                        emit_avs(j - 1)
                    if j < len(DEFERRED):
                        kind, idx = DEFERRED[j]
                        proj_chain(pmix, kind, 3, idx)
                    if j >= 2:
                        emit_recmul(j - 2)
                    emit_exps(j)
                    emit_maskmuls(j)
                    if j >= 1:
                        emit_den(j - 1)
                        emit_attcopies(j - 1)
                    for (stq, et) in OSCHED.get(j, ()):
                        oproj_unit(stq, et)
                # flush
                emit_avs(31)
                emit_den(31)
                emit_attcopies(31)
                emit_bc(30)
                emit_recmul(30)
                emit_bc(31)
                emit_recmul(31)
                for et in range(8):
                    oproj_unit(3, et)

    nc.compile()
    return nc


def _host_masks():
    p = np.arange(128)[:, None]

    def band(delta, lo, w):
        sl = np.arange(w)[None, :]
        return (np.abs(delta + p - lo - sl) <= 32).astype(np.float32)

    def gcols(w):
        sl = np.arange(w)[None, :]
        return ((p < 4) + 0 * sl).astype(np.float32)

    def special(w):  # r=0 j0: t in [0,128), s=sl
        sl = np.arange(w)[None, :]
        return ((sl >= 4) & ((np.abs(p - sl) <= 32) | (p < 4))).astype(np.float32)

    interior = np.concatenate(
        [gcols(256), band(-128, 0, 32), band(0, 0, 160),
         band(128, 96, 160), band(256, 224, 32)], axis=1)
    r0 = np.concatenate(
        [special(256), band(128, 96, 160), band(256, 224, 32)], axis=1)
    r7 = np.concatenate(
        [gcols(256), band(-128, 0, 32), band(0, 0, 160),
         band(128, 96, 160)], axis=1)
    full = np.concatenate([interior, r0, r7], axis=1)
    assert full.shape == (128, MASK_W)
    return full.astype(ml_dtypes.bfloat16)


_NC = None
_LAST_IN_MAPS = None


def kernel(x, Wq, bq, Wk, bk, Wv, bv, Wo, bo):
    global _NC
    if _NC is None:
        _NC = build_nc()
    nc = _NC
    x = np.asarray(x, np.float32)
    B = x.shape[0]
    bf = ml_dtypes.bfloat16

    def chunked_T(a):  # [R, C] -> [128, C//128, R]; [p, c, r] = a[r, 128c+p]
        at = np.ascontiguousarray(a.T)
        return at.reshape(at.shape[0] // 128, 128, at.shape[1]).transpose(1, 0, 2)

    masks_h = _host_masks()
    in_maps = []
    for core in range(NCORES):
        b, g = core // 2, core % 2
        gs = slice(512 * g, 512 * (g + 1))
        in_maps.append({
            "xT": np.ascontiguousarray(chunked_T(x[b])).astype(bf),
            "wq": np.ascontiguousarray(chunked_T(np.asarray(Wq)[gs, :])).astype(bf),
            "wk": np.ascontiguousarray(chunked_T(np.asarray(Wk)[gs, :])).astype(bf),
            "wv": np.ascontiguousarray(chunked_T(np.asarray(Wv)[gs, :])).astype(bf),
            "wo": np.ascontiguousarray(chunked_T(np.asarray(Wo)[:, gs])).astype(bf),
            "bq_c": np.asarray(bq)[gs].reshape(4, 128).T.copy().astype(np.float32),
            "bk_c": np.asarray(bk)[gs].reshape(4, 128).T.copy().astype(np.float32),
            "bv_b": np.broadcast_to(
                np.asarray(bv)[gs], (128, 512)).copy().astype(np.float32),
            "masks": masks_h,
        })

    global _LAST_IN_MAPS
    _LAST_IN_MAPS = in_maps
    res = run_bass_kernel_spmd(nc, in_maps, list(range(NCORES)))
    out = np.empty((B, S, D), np.float32)
    for b in range(B):
        acc = res.results[2 * b]["out"].astype(np.float32) + \
            res.results[2 * b + 1]["out"].astype(np.float32)
        full_T = acc.transpose(1, 0, 2).reshape(D, S)
        out[b] = full_T.T + np.asarray(bo)[None, :]
    return out


# revision 26
# speedup vs baseline: 1.0527x; 1.0527x over previous
"""BlockSparseAttention TRN2 kernel — 8-core SPMD (batch x head-half sharding).

Reference computation (B=4, S=2048, D=1024, H=16, Dh=64):
  q/k/v = x @ W{q,k,v}.T + b   -> [B,H,S,Dh]
  scores = q k^T / 8, masked to |i-j|<=32 plus global rows/cols (<4)
  out = softmax(scores) v  -> reassembled -> @ Wo.T + bo

Sharding: core c handles batch b=c//2, head-group g=c%2 (heads 8g..8g+7,
channels 512g..512g+511). Each core computes its heads' attention output and
a PARTIAL o-projection (contraction over its 512 channels); host sums the two
partials per batch and adds bo.

On-chip layout is fully transposed (d-major): projections compute q_T/k_T
directly as [chan, s]; scores are computed transposed [t, s] so softmax sums
land in a matmul ones-row (v augmented with a ones column).

v2 design notes (vs v1):
  - x is SBUF-resident, DMA'd once; projections run cb-outer so q/k/v fit in
    a 4-bank PSUM rotation and form one continuous dense PE stream.
  - Attention is a 1-deep software pipeline over the 32 (r, hp) pairs:
    scores(j) | bc(j-2) | AV(j-1) per iteration, with PE work grouped by
    tiling mode (64-row scores w/ h0/h64 row-tile concurrency, 32-row
    broadcast MMs, 128-row AVs) to minimize PE array mode-switch drains.
  - Global (t<4) key columns are computed as a 5th band-style chunk with a
    0/1 bf16 mask MULTIPLY after exp (no -1e9 add pass, no 4-partition strip
    matmuls, no 32-row AV section).
  - Softmax normalization: denominator row -> bf16 copy -> PE K=1 broadcast
    matmul (col-tiled into the att layout) -> reciprocal_approx_fast on the
    [128, 256] broadcast -> one [128, 256] multiply. No GpSimd broadcast.
  - o-projection emitted as dense N=512 blocks after the norm of each s-tile
    pair; output stored bf16 (host sums partials in f32).
"""
import numpy as np
import ml_dtypes

import concourse.bass as bass
import concourse.bacc as bacc
import concourse.tile as tile
import concourse.mybir as mybir
from concourse.bass_utils import run_bass_kernel_spmd

F32 = mybir.dt.float32
BF16 = mybir.dt.bfloat16
AF = mybir.ActivationFunctionType
ALU = mybir.AluOpType

S = 2048
D = 1024
NCORES = 8
SCALE = 0.125

# ---------------------------------------------------------------------------
# Chunk plans: per s-tile r (256 rows), the score/AV chunks.
# Each chunk: (sc, exoff, lo, w, mkind) where sc = 128-row v/t chunk index,
# exoff = column offset in the packed ex tile, lo = s-local start, w = width,
# mkind identifies the mask pattern.
#   mkind: 'G' (global cols t<4), 'D' (delta=-128), 'A' (delta=0),
#          'B' (delta=+128), 'C' (delta=+256), 'S' (r=0 special j0)
# One sct PSUM tile per (pair, hs) holds all packed score columns (<=448).
# ---------------------------------------------------------------------------


# v5 chunks: each entry (kind, idx, exoff, lo, w).
#   kind 'al': 128-aligned chunk — stationary k_sb[.., idx*128:+128] /
#              v_sb[:, idx, ..] (r=0 only, plus the gsc/gAV global-row path)
#   kind 'g':  global-prepended chunk — stationary kg_sb[.., idx, :] /
#              vg_sb[:, idx, ..]: cols/rows 0:4 are the t<4 global keys,
#              4:128 are band keys from T0(idx).
# T0 per g-chunk index: r in 1..6: 256r-64, 256r+60, 256r+184 at
# idx 3(r-1)+{0,1,2}; r=7: 1728, 1852, 1924.


def chunk_T0(c):
    r, i = c // 3 + 1, c % 3
    if r == 7:
        return (1728, 1852, 1924)[i]
    return 256 * r + (-64, 60, 184)[i]


N_GCHUNKS = 21


def chunk_plan(r):
    if r == 0:
        return [('al', 0, 0, 0, 256), ('al', 1, 256, 96, 160),
                ('al', 2, 416, 224, 32)]
    b = 3 * (r - 1)
    return [('g', b, 0, 0, 92), ('g', b + 1, 92, 28, 188),
            ('g', b + 2, 280, 152, 104)]


CHUNKS = [chunk_plan(r) for r in range(8)]
WR = [sum(c[4] for c in CHUNKS[r]) for r in range(8)]  # 448 / 384
MOFF = {0: 384, 7: 832}  # interior at 0
MASK_W = 384 + 448 + 384


def mask_off(r):
    return MOFF.get(r, 0)


def build_nc():
    nc = bacc.Bacc()
    xT = nc.dram_tensor("xT", [128, 8, S], BF16, kind="ExternalInput")
    wq = nc.dram_tensor("wq", [128, 8, 512], BF16, kind="ExternalInput")
    wk = nc.dram_tensor("wk", [128, 8, 512], BF16, kind="ExternalInput")
    wv = nc.dram_tensor("wv", [128, 8, 512], BF16, kind="ExternalInput")
    wo = nc.dram_tensor("wo", [128, 4, 1024], BF16, kind="ExternalInput")
    bq_c = nc.dram_tensor("bq_c", [128, 4], F32, kind="ExternalInput")
    bk_c = nc.dram_tensor("bk_c", [128, 4], F32, kind="ExternalInput")
    bv_b = nc.dram_tensor("bv_b", [128, 512], F32, kind="ExternalInput")
    masks = nc.dram_tensor("masks", [128, MASK_W], BF16, kind="ExternalInput")
    out = nc.dram_tensor("out", [128, 8, S], BF16, kind="ExternalOutput")

    with tile.TileContext(nc) as tc:
        with (
            tc.tile_pool(name="pers", bufs=1) as pers,
            tc.tile_pool(name="small", bufs=1) as small,
        ):
            x_sb = pers.tile([128, 8, S], BF16)
            q_sb = pers.tile([128, 4, S], BF16)
            k_sb = pers.tile([128, 4, S], BF16)
            v_sb = pers.tile([128, 16, 520], BF16)
            kg_sb = pers.tile([128, 4, N_GCHUNKS, 128], BF16)
            vg_sb = pers.tile([128, N_GCHUNKS, 520], BF16)
            att_sb = pers.tile([128, 4, S], BF16)
            masks_sb = pers.tile([128, MASK_W], BF16)
            wq_sb = pers.tile([128, 8, 512], BF16)
            wk_sb = pers.tile([128, 8, 512], BF16)
            wv_sb = pers.tile([128, 8, 512], BF16)
            wo_sb = pers.tile([128, 4, 1024], BF16)
            bq_sb = small.tile([128, 4], F32)
            bk_sb = small.tile([128, 4], F32)
            bv_sb = small.tile([128, 512], F32)
            ones_sb = small.tile([1, 128], BF16)
            nc.vector.memset(ones_sb, 1.0)

            # Two HWDGE queues in parallel: weights/biases/masks on the
            # scalar-engine queue, x on the sync queue — first q chain can
            # start once wq and x-st0 land (~6us).
            nc.scalar.dma_start(out=wq_sb, in_=wq.ap())
            nc.scalar.dma_start(out=bq_sb, in_=bq_c.ap())
            nc.scalar.dma_start(out=bk_sb, in_=bk_c.ap())
            nc.scalar.dma_start(out=wk_sb, in_=wk.ap())
            nc.scalar.dma_start(out=wv_sb, in_=wv.ap())
            nc.scalar.dma_start(out=bv_sb, in_=bv_b.ap())
            nc.scalar.dma_start(out=masks_sb, in_=masks.ap())
            nc.scalar.dma_start(out=wo_sb, in_=wo.ap())
            for st in range(4):
                ssl = slice(st * 512, (st + 1) * 512)
                for dc in range(8):
                    nc.sync.dma_start(out=x_sb[:, dc, ssl], in_=xT.ap()[:, dc, ssl])

            # ---------------- Phase 1: projections (dense, x-resident) -----
            # st=0..2 here; st=3's 12 chains are deferred into the early
            # attention iterations as dense HAM-warming filler.
            def proj_chain(pool, kind, st, idx):
                ssl = slice(st * 512, (st + 1) * 512)
                if kind == 'q' or kind == 'k':
                    csl = slice(idx * 128, (idx + 1) * 128)
                    w_sb = wq_sb if kind == 'q' else wk_sb
                    dst = q_sb if kind == 'q' else k_sb
                    b_sb = bq_sb if kind == 'q' else bk_sb
                    pq = pool.tile([128, 512], F32, tag=pool_tag(pool),
                                   name=f"p{kind}{st}_{idx}")
                    for dc in range(8):
                        nc.tensor.matmul(
                            pq, w_sb[:, dc, csl], x_sb[:, dc, ssl],
                            start=(dc == 0), stop=(dc == 7))
                    nc.vector.tensor_scalar(
                        out=dst[:, idx, ssl], in0=pq,
                        scalar1=b_sb[:, idx:idx + 1], scalar2=None,
                        op0=ALU.add)
                else:  # 'v'
                    xsl = slice(st * 512 + idx * 128, st * 512 + idx * 128 + 128)
                    pv = pool.tile([128, 512], F32, tag=pool_tag(pool),
                                   name=f"pv{st}_{idx}")
                    for dc in range(8):
                        nc.tensor.matmul(
                            pv, x_sb[:, dc, xsl], wv_sb[:, dc, :],
                            start=(dc == 0), stop=(dc == 7))
                    sc = st * 4 + idx
                    vview = v_sb[:, sc, :].rearrange("p (h w) -> p h w", h=8)
                    nc.vector.tensor_add(
                        out=vview[:, :, 0:64],
                        in0=pv.rearrange("p (h w) -> p h w", h=8),
                        in1=bv_sb.rearrange("p (h w) -> p h w", h=8))
                    nc.vector.memset(vview[:, :, 64:65], 1.0)

            def pool_tag(pool):
                return "proj" if pool is not None and pool.name == "pproj" \
                    else "pobc"

            def emit_kg(c):
                T0 = chunk_T0(c)
                for hp in range(4):
                    nc.vector.tensor_copy(out=kg_sb[:, hp, c, 0:4],
                                          in_=k_sb[:, hp, 0:4])
                    nc.vector.tensor_copy(out=kg_sb[:, hp, c, 4:128],
                                          in_=k_sb[:, hp, T0:T0 + 124])

            def emit_vg(c):
                # SBUF->SBUF DMA: arbitrary partition shifts (Scalar/Vector
                # engines require quadrant-aligned output partition bases)
                T0 = chunk_T0(c)
                m, sc0 = T0 % 128, T0 // 128
                nc.sync.dma_start(out=vg_sb[0:4, c, :], in_=v_sb[0:4, 0, :])
                n1 = 128 - m
                nc.sync.dma_start(out=vg_sb[4:4 + n1, c, :],
                                  in_=v_sb[m:128, sc0, :])
                if n1 < 124:
                    nc.sync.dma_start(out=vg_sb[4 + n1:128, c, :],
                                      in_=v_sb[0:124 - n1, sc0 + 1, :])

            # chunks whose k/v sources live entirely in st<=2 (t < 1536)
            GCHUNKS_A = [c for c in range(N_GCHUNKS) if chunk_T0(c) + 124 <= 1536]
            GCHUNKS_B = [c for c in range(N_GCHUNKS) if chunk_T0(c) + 124 > 1536]

            with tc.tile_pool(name="pproj", bufs=4, space="PSUM") as pproj:
                for st in range(3):
                    for cb in range(4):
                        proj_chain(pproj, 'q', st, cb)
                    for cb in range(4):
                        proj_chain(pproj, 'k', st, cb)
                    for s4 in range(4):
                        proj_chain(pproj, 'v', st, s4)
                for c in GCHUNKS_A:
                    emit_kg(c)
                    emit_vg(c)

            # ---------------- Phase 2: attention pipeline ------------------
            with (
                tc.tile_pool(name="psct", bufs=2, space="PSUM") as psct,
                tc.tile_pool(name="paug", bufs=3, space="PSUM") as paug,
                # bc (normalizer broadcast), po (o-proj) and the deferred
                # st=3 projection chains share one 3-slot rotation:
                # 2 + 3 + 3 = 8 PSUM banks total.
                tc.tile_pool(name="pmix", bufs=3, space="PSUM") as pmix,
                tc.tile_pool(name="epool", bufs=4) as epool,
                tc.tile_pool(name="rpool", bufs=2) as rpool,
                tc.tile_pool(name="opool", bufs=3) as opool,
            ):
                # r=0 needs full k (gsc) and full v (gAV): schedule it after
                # r=1..3 so the deferred st=3 chains (emitted in iterations
                # 0-11) complete first.
                PAIRS = [(r, hp) for r in (1, 2, 3, 0, 4, 5, 6, 7)
                         for hp in range(4)]
                # deferred st=3 projection chains: one per early iteration
                DEFERRED = [('k', cb) for cb in range(4)] + \
                    [('q', cb) for cb in range(4)] + \
                    [('v', s4) for s4 in range(4)]
                # oproj availability: st1 after iter 13 (r2,r3 normed),
                # st0 after 17, st2 after 25, st3 in flush. Every iteration
                # from 13 gets at least one dense unit (HAM warmth).
                OSCHED = {}
                for i, e in enumerate(range(13, 17)):
                    OSCHED[e] = [(1, i)]
                for i, e in enumerate(range(17, 21)):
                    OSCHED[e] = [(1, 4 + i), (0, i)]
                for i, e in enumerate(range(21, 25)):
                    OSCHED[e] = [(0, 4 + i)]
                for i, e in enumerate(range(25, 29)):
                    OSCHED[e] = [(2, i)]
                OSCHED[29] = [(2, 4), (2, 5)]
                OSCHED[30] = [(2, 6), (2, 7)]
                state = {}  # j -> dict of tiles

                def emit_scores(j):
                    r, hp = PAIRS[j]
                    st = {}
                    wr = WR[r]
                    for hs in (0, 64):
                        st[f"sct{hs}"] = psct.tile(
                            [128, 512], F32, tag="sct", name=f"sc{j}_{hs}")
                    # interleave h0/h64 chunk by chunk: row-tile concurrency
                    nch = len(CHUNKS[r])
                    for ci, (kind, ix, exoff, lo, w) in enumerate(CHUNKS[r]):
                        for hs in (0, 64):
                            if kind == 'al':
                                kst = k_sb[hs:hs + 64, hp,
                                           ix * 128:ix * 128 + 128]
                            else:
                                kst = kg_sb[hs:hs + 64, hp, ix, :]
                            nc.tensor.matmul(
                                st[f"sct{hs}"][:, exoff:exoff + w],
                                kst,
                                q_sb[hs:hs + 64, hp,
                                     r * 256 + lo:r * 256 + lo + w],
                                start=(ci == 0), stop=(ci == nch - 1),
                                skip_group_check=True)
                    if r == 0:
                        for hs in (0, 64):
                            st[f"gsc{hs}"] = psct.tile(
                                [128, 64], F32, tag="sct", name=f"gs{j}_{hs}")
                        for kk in range(16):
                            for hs in (0, 64):
                                nc.tensor.matmul(
                                    st[f"gsc{hs}"][:, 4 * kk:4 * kk + 4],
                                    k_sb[hs:hs + 64, hp,
                                         128 * kk:128 * kk + 128],
                                    q_sb[hs:hs + 64, hp, 0:4],
                                    start=(kk == 0), stop=(kk == 15),
                                    skip_group_check=True)
                    state[j] = st

                def emit_exps(j):
                    r, hp = PAIRS[j]
                    st = state[j]
                    wr = WR[r]
                    for hs in (0, 64):
                        ex = epool.tile([128, 448], BF16, tag="ex",
                                        name=f"ex{j}_{hs}")
                        st[f"ex{hs}"] = ex
                        nc.scalar.activation(
                            ex[:, 0:wr], st[f"sct{hs}"][:, 0:wr],
                            AF.Exp, scale=SCALE)
                        if r == 0:
                            exg = epool.tile([128, 64], BF16, tag="exg",
                                             name=f"xg{j}_{hs}")
                            st[f"exg{hs}"] = exg
                            nc.scalar.activation(exg, st[f"gsc{hs}"],
                                                 AF.Exp, scale=SCALE)

                def emit_maskmuls(j):
                    r, hp = PAIRS[j]
                    st = state[j]
                    wr = WR[r]
                    mo = mask_off(r)
                    for hs in (0, 64):
                        nc.gpsimd.tensor_mul(
                            out=st[f"ex{hs}"][:, 0:wr],
                            in0=st[f"ex{hs}"][:, 0:wr],
                            in1=masks_sb[:, mo:mo + wr])

                def emit_avs(j):
                    r, hp = PAIRS[j]
                    st = state[j]
                    aug = paug.tile([65, 512], F32, tag="aug", name=f"au{j}")
                    st["aug"] = aug
                    n_ch = len(CHUNKS[r])
                    for hi, hs in enumerate((0, 64)):
                        half = (hs // 64) * 256
                        h65 = (hp * 2 + hs // 64) * 65
                        ex = st[f"ex{hs}"]
                        for ci, (kind, ix, exoff, lo, w) in enumerate(CHUNKS[r]):
                            if kind == 'al':
                                vst = v_sb[:, ix, h65:h65 + 65]
                            else:
                                vst = vg_sb[:, ix, h65:h65 + 65]
                            last = (r != 0 and hi == 1 and ci == n_ch - 1)
                            nc.tensor.matmul(
                                aug[:, half + lo:half + lo + w],
                                vst,
                                ex[:, exoff:exoff + w],
                                start=(hi == 0 and ci == 0), stop=last,
                                skip_group_check=True)
                    if r == 0:
                        for hi, hs in enumerate((0, 64)):
                            half = (hs // 64) * 256
                            h65 = (hp * 2 + hs // 64) * 65
                            exg = st[f"exg{hs}"]
                            for kk in range(16):
                                nc.tensor.matmul(
                                    aug[:, half:half + 4],
                                    v_sb[:, kk, h65:h65 + 65],
                                    exg[:, 4 * kk:4 * kk + 4],
                                    start=False,
                                    stop=(hi == 1 and kk == 15),
                                    skip_group_check=True)

                def emit_den(j):
                    st = state[j]
                    den = epool.tile([1, 512], BF16, tag="den", name=f"dn{j}")
                    st["den"] = den
                    nc.scalar.copy(out=den, in_=st["aug"][64:65, :])

                def emit_attcopies(j):
                    r, hp = PAIRS[j]
                    st = state[j]
                    rsl = slice(r * 256, (r + 1) * 256)
                    for hs in (0, 64):
                        half = (hs // 64) * 256
                        nc.scalar.copy(out=att_sb[hs:hs + 64, hp, rsl],
                                       in_=st["aug"][0:64, half:half + 256])

                def emit_bc(j):
                    st = state[j]
                    bc = pmix.tile([128, 256], F32, tag="pobc", name=f"bc{j}")
                    st["bc"] = bc
                    den = st["den"]
                    nc.tensor.matmul(bc[0:64, :], ones_sb[0:1, 0:64],
                                     den[0:1, 0:256], start=True, stop=True)
                    nc.tensor.matmul(bc[64:128, :], ones_sb[0:1, 0:64],
                                     den[0:1, 256:512], start=True, stop=True,
                                     tile_position=(0, 64))
                    del st["den"]

                def emit_recmul(j):
                    r, hp = PAIRS[j]
                    st = state[j]
                    rsl = slice(r * 256, (r + 1) * 256)
                    rec = rpool.tile([128, 256], F32, tag="rec", name=f"rc{j}")
                    nc.vector.reciprocal_approx_fast(out=rec, in_=st["bc"])
                    nc.vector.tensor_mul(
                        out=att_sb[:, hp, rsl], in0=att_sb[:, hp, rsl],
                        in1=rec)
                    state.pop(j, None)

                def oproj_unit(stq, et):
                    # one dense 4-MM N=512 chain: HAM-warming filler spread
                    # through the attention pipeline
                    ssl = slice(stq * 512, (stq + 1) * 512)
                    esl = slice(et * 128, (et + 1) * 128)
                    po = pmix.tile([128, 512], F32, tag="pobc",
                                   name=f"po{stq}_{et}")
                    for cb in range(4):
                        nc.tensor.matmul(
                            po, wo_sb[:, cb, esl], att_sb[:, cb, ssl],
                            start=(cb == 0), stop=(cb == 3))
                    otq = opool.tile([128, 512], BF16, tag="otq",
                                     name=f"otq{stq}_{et}")
                    nc.vector.tensor_copy(out=otq, in_=po)
                    nc.sync.dma_start(out=out.ap()[:, et, ssl], in_=otq)

                for j in range(32):
                    emit_scores(j)
                    if j >= 2:
                        emit_bc(j - 2)
                    if j >= 1:
                        emit_avs(j - 1)
                    if j < len(DEFERRED):
                        kind, idx = DEFERRED[j]
                        proj_chain(pmix, kind, 3, idx)
                    if j == 4:
                        for c in GCHUNKS_B:
                            emit_kg(c)
                    if 12 <= j <= 14:
                        for c in GCHUNKS_B[(j - 12) * 3:(j - 11) * 3]:
                            emit_vg(c)
                    if j >= 2:
                        emit_recmul(j - 2)
                    emit_exps(j)
                    emit_maskmuls(j)
                    if j >= 1:
                        emit_den(j - 1)
                        emit_attcopies(j - 1)
                    for (stq, et) in OSCHED.get(j, ()):
                        oproj_unit(stq, et)
                # flush
                emit_avs(31)
                emit_den(31)
                emit_attcopies(31)
                emit_bc(30)
                emit_recmul(30)
                emit_bc(31)
                emit_recmul(31)
                for et in range(8):
                    oproj_unit(3, et)

    nc.compile()
    return nc


def _host_masks():
    p = np.arange(128)[:, None]

    def ME1(w=92):  # owns globals for s' in [0,92)
        sl = np.arange(w)[None, :]
        return np.where(p < 4, 1.0,
                        (np.abs(p - 68 - sl) <= 32)).astype(np.float32)

    def ME2(w):  # globals from sl>=64
        sl = np.arange(w)[None, :]
        return np.where(p < 4, (sl >= 64),
                        (np.abs(p + 28 - sl) <= 32)).astype(np.float32)

    def ME3r7(w=104):  # clamped T0=1924; exclude overlap with E2 (t<1976)
        sl = np.arange(w)[None, :]
        band = (np.abs(p - 24 - sl) <= 32) & ((p >= 56) | (sl >= 64))
        return np.where(p < 4, (sl >= 64), band).astype(np.float32)

    def band(delta, lo, w):
        sl = np.arange(w)[None, :]
        return (np.abs(delta + p - lo - sl) <= 32).astype(np.float32)

    def special(w=256):  # r=0 j0: t in [0,128), s=sl
        sl = np.arange(w)[None, :]
        return ((sl >= 4) & ((np.abs(p - sl) <= 32) | (p < 4))).astype(np.float32)

    interior = np.concatenate([ME1(), ME2(188), ME2(104)], axis=1)
    r0 = np.concatenate(
        [special(), band(128, 96, 160), band(256, 224, 32)], axis=1)
    r7 = np.concatenate([ME1(), ME2(188), ME3r7()], axis=1)
    full = np.concatenate([interior, r0, r7], axis=1)
    assert full.shape == (128, MASK_W)
    return full.astype(ml_dtypes.bfloat16)


_NC = None
_LAST_IN_MAPS = None


def kernel(x, Wq, bq, Wk, bk, Wv, bv, Wo, bo):
    global _NC
    if _NC is None:
        _NC = build_nc()
    nc = _NC
    x = np.asarray(x, np.float32)
    B = x.shape[0]
    bf = ml_dtypes.bfloat16

    def chunked_T(a):  # [R, C] -> [128, C//128, R]; [p, c, r] = a[r, 128c+p]
        at = np.ascontiguousarray(a.T)
        return at.reshape(at.shape[0] // 128, 128, at.shape[1]).transpose(1, 0, 2)

    masks_h = _host_masks()
    in_maps = []
    for core in range(NCORES):
        b, g = core // 2, core % 2
        gs = slice(512 * g, 512 * (g + 1))
        in_maps.append({
            "xT": np.ascontiguousarray(chunked_T(x[b])).astype(bf),
            "wq": np.ascontiguousarray(chunked_T(np.asarray(Wq)[gs, :])).astype(bf),
            "wk": np.ascontiguousarray(chunked_T(np.asarray(Wk)[gs, :])).astype(bf),
            "wv": np.ascontiguousarray(chunked_T(np.asarray(Wv)[gs, :])).astype(bf),
            "wo": np.ascontiguousarray(chunked_T(np.asarray(Wo)[:, gs])).astype(bf),
            "bq_c": np.asarray(bq)[gs].reshape(4, 128).T.copy().astype(np.float32),
            "bk_c": np.asarray(bk)[gs].reshape(4, 128).T.copy().astype(np.float32),
            "bv_b": np.broadcast_to(
                np.asarray(bv)[gs], (128, 512)).copy().astype(np.float32),
            "masks": masks_h,
        })

    global _LAST_IN_MAPS
    _LAST_IN_MAPS = in_maps
    res = run_bass_kernel_spmd(nc, in_maps, list(range(NCORES)))
    out = np.empty((B, S, D), np.float32)
    for b in range(B):
        acc = res.results[2 * b]["out"].astype(np.float32) + \
            res.results[2 * b + 1]["out"].astype(np.float32)
        full_T = acc.transpose(1, 0, 2).reshape(D, S)
        out[b] = full_T.T + np.asarray(bo)[None, :]
    return out
